# revision 6
# baseline (speedup 1.0000x reference)
"""DenseCRF Gaussian-kernel loss on 8 TRN2 NeuronCores.

loss = -W/N * sum_n sum_ij exp(-0.5||f_i-f_j||^2) * (S^T S)_ij

Decomposition (per image, P=6400 pixels, f in R^5):
  exp arg  = f_i.f_j - 0.5|f_i|^2 - 0.5|f_j|^2   -> one 9-deep fp16 matmul
             (rows: 5 features + ones + hi/lo split of -0.5|f|^2), so the
             exp needs no per-row bias and any row-block shares one ACT op.
  sum_ij W_ij G_ij = sum_k (S_k^T W S_k): per 128x512 tile W, accumulate
             T[16,512] += (w * S_rows)^T @ W on the PE, then one fused
             multiply-reduce against S_cols on the DVE.
  Symmetry W_ij = W_ji: strictly-upper tiles weighted x2 (folded into the
             S^T matmul weights), diagonal 4x4-block squares weighted x1.

Sharding: cores 2n, 2n+1 split image n's upper triangle; all cores run one
identical program over host-packed per-core operands (dummy zero-S tiles pad
the schedule). Each core returns a [16,13] partial-sum block; host reduces.
"""

import os

# The Bass program executes through jax/PJRT on the axon-tunneled TRN2 cores;
# a JAX_PLATFORMS=cpu pin (common for running the jax reference) would silently
# reroute execution to a fake NRT.  Clear it before jax initializes.
if os.environ.get("JAX_PLATFORMS") == "cpu":
    del os.environ["JAX_PLATFORMS"]

import numpy as np
import ml_dtypes

import concourse.bacc as bacc
import concourse.bass as bass  # noqa: F401  (bass types used via bacc/tile)
import concourse.mybir as mybir
import concourse.tile as tile
from concourse.bass_utils import run_bass_kernel_spmd

N_IMG, K_CLS, H_IN, W_IN = 4, 16, 160, 160
HO = WO = 80
P = HO * WO               # 6400 pixels after 2x downscale
SIGMA_RGB = 15.0
SIGMA_XY = 50.0           # 100 * scale_factor 0.5
LOSS_WEIGHT = 2e-9
NCORES = 8

RBLK = 128                # row-block (PE partition dim)
CW = 512                  # column-group width (one PSUM bank of fp32)
NG = 13                   # column groups; group 12 covers cols 6144:6656 (256 pad)
PJ = NG * CW              # 6656 padded column extent
MEGAS = [g + 1 for g in range(NG)]      # per-core megas (2 tiles each) per group
NTILE = 2 * sum(MEGAS)    # 182 tiles of [128,512] per core

_dt = mybir.dt
_BF16 = ml_dtypes.bfloat16


def _pool2x2(x):
    # torch bilinear align_corners=False at exact 2x = 2x2 average, in the
    # reference's evaluation order.
    r = x[..., 0::2, :] * 0.5 + x[..., 1::2, :] * 0.5
    return r[..., 0::2] * 0.5 + r[..., 1::2] * 0.5


def _features(img):
    """img [3,160,160] f32 -> (AI [9,P] f16, BJ [9,P] f16).

    exp arg for pair (i,j) = sum_d AI[d,i]*BJ[d,j] computed by the PE:
      AI = [f(5), 1, 1, shi, slo];  BJ = [f(5), shi, slo, 1, 1]
    with shi+slo an fp16 hi/lo split of -0.5|f|^2 so the diagonal cancels to
    ~1e-4 even though single fp16 could only hold it to ~0.2.
    """
    sub = img[:, ::2, ::2]                                  # nearest resize
    rgb = sub.reshape(3, P) / SIGMA_RGB
    yy, xx = np.meshgrid(np.arange(HO, dtype=np.float32),
                         np.arange(WO, dtype=np.float32), indexing="ij")
    pos = np.stack([xx.ravel(), yy.ravel()]) / SIGMA_XY     # [2,P]
    f16 = np.concatenate([pos, rgb], 0).astype(np.float16)  # [5,P]
    fh = f16.astype(np.float64)
    s = -0.5 * (fh * fh).sum(0)                             # [P] exact
    shi = s.astype(np.float16)
    slo = (s - shi.astype(np.float64)).astype(np.float16)
    ones = np.ones(P, np.float16)
    AI = np.concatenate([f16, ones[None], ones[None], shi[None], slo[None]])
    BJ = np.concatenate([f16, shi[None], slo[None], ones[None], ones[None]])
    return AI, BJ


def _image_tile_groups():
    """Per column-group list of (row_block, weight) or None (pad) tiles.

    Group C<12 covers cols [512C, 512C+512): strictly-upper rows r<4C at x2
    plus the diagonal 4x4-block square rows 4C..4C+3 at x1.  Group 12 covers
    cols [6144, 6656) (256 real + 256 zero-S pad): rows r<48 at x2 plus the
    remainder square rows 48,49 at x1, padded to an even 52 tiles.
    """
    groups = []
    for C in range(12):
        groups.append([(r, 2.0) for r in range(4 * C)]
                      + [(r, 1.0) for r in range(4 * C, 4 * C + 4)])
    groups.append([(r, 2.0) for r in range(48)]
                  + [(48, 1.0), (49, 1.0), None, None])
    return groups


def _core_schedule(half):
    """Flat per-core tile list [(group, row_block, weight) or None] in program
    order: for each group g, its megas are the image group's mega list
    (consecutive tile pairs) taken with stride 2 starting at `half`."""
    sched = []
    for g, tiles in enumerate(_image_tile_groups()):
        megas = [tiles[2 * m: 2 * m + 2] for m in range(len(tiles) // 2)]
        for mega in megas[half::2]:
            for t in mega:
                sched.append(None if t is None else (g, t[0], t[1]))
    assert len(sched) == NTILE
    return sched


def _pack_core(AI, BJ, S, half):
    """Build one core's input arrays for its half of one image's triangle."""
    sched = _core_schedule(half)
    aip = np.zeros((9, NTILE * 128), np.float16)
    sitp = np.zeros((128, NTILE * 16), np.float32)
    for t, ent in enumerate(sched):
        if ent is None:
            continue
        g, r, w = ent
        aip[:, t * 128:(t + 1) * 128] = AI[:, r * 128:(r + 1) * 128]
        sitp[:, t * 16:(t + 1) * 16] = w * S[:, r * 128:(r + 1) * 128].T
    bjp = np.zeros((9, PJ), np.float16)
    bjp[:, :P] = BJ
    bjp[5, P:] = np.float16(-50.0)   # pad cols: exp(-50-0.5|f_i|^2), no inf
    sjp = np.zeros((16, PJ), np.float32)
    sjp[:, :P] = S
    return {
        "AIP": aip,
        "BJP": bjp,
        "SITP": sitp.astype(_BF16),
        "SJP": sjp,
    }


def build_inputs(images, segmentations):
    """FULL inputs -> per-core in_maps (cores 2n, 2n+1 share image n)."""
    in_maps = []
    for n in range(N_IMG):
        AI, BJ = _features(np.asarray(images[n], np.float32))
        S = _pool2x2(np.asarray(segmentations[n], np.float32)).reshape(K_CLS, P)
        for half in range(2):
            in_maps.append(_pack_core(AI, BJ, S, half))
    return in_maps


def build_program(repeat=1):
    # repeat>1 re-runs the (idempotent) compute body back-to-back inside one
    # NEFF — used only by the benchmark to difference away dispatch overhead.
    nc = bacc.Bacc("TRN2", target_bir_lowering=False, debug=False)
    aip_d = nc.dram_tensor("AIP", (9, NTILE * 128), _dt.float16, kind="ExternalInput")
    bjp_d = nc.dram_tensor("BJP", (9, PJ), _dt.float16, kind="ExternalInput")
    sitp_d = nc.dram_tensor("SITP", (128, NTILE * 16), _dt.bfloat16, kind="ExternalInput")
    sjp_d = nc.dram_tensor("SJP", (16, PJ), _dt.float32, kind="ExternalInput")
    acc_d = nc.dram_tensor("ACC", (16, NG), _dt.float32, kind="ExternalOutput")

    with tile.TileContext(nc) as tc:
        with (
            tc.tile_pool(name="const", bufs=1) as cpool,
            tc.tile_pool(name="w", bufs=4) as wpool,
            tc.tile_pool(name="red", bufs=2) as rpool,
            tc.tile_pool(name="xps", bufs=3, space="PSUM") as xpool,
            tc.tile_pool(name="tps", bufs=2, space="PSUM") as tpool,
        ):
            AIP = cpool.tile([9, NTILE * 128], _dt.float16)
            BJP = cpool.tile([9, PJ], _dt.float16)
            SITP = cpool.tile([128, NTILE * 16], _dt.bfloat16)
            SJP = cpool.tile([16, PJ], _dt.float32)
            ACC = cpool.tile([16, NG], _dt.float32)
            nc.sync.dma_start(AIP[:], aip_d[:])
            nc.sync.dma_start(BJP[:], bjp_d[:])
            nc.sync.dma_start(SITP[:], sitp_d[:])
            nc.sync.dma_start(SJP[:], sjp_d[:])

            for _rep in range(repeat):
                t_idx = 0
                for g in range(NG):
                    mg = MEGAS[g]
                    T = tpool.tile([16, CW], _dt.float32)
                    for m in range(mg):
                        x = xpool.tile([128, 2 * CW], _dt.float32)
                        for h in range(2):
                            nc.tensor.matmul(
                                x[:, h * CW:(h + 1) * CW],
                                AIP[:, (t_idx + h) * 128:(t_idx + h + 1) * 128],
                                BJP[:, g * CW:(g + 1) * CW],
                                start=True, stop=True,
                            )
                        w = wpool.tile([128, 2 * CW], _dt.bfloat16)
                        nc.scalar.activation(w[:], x[:], mybir.ActivationFunctionType.Exp)
                        for h in range(2):
                            nc.tensor.matmul(
                                T[:],
                                SITP[:, (t_idx + h) * 16:(t_idx + h + 1) * 16],
                                w[:, h * CW:(h + 1) * CW],
                                start=(m == 0 and h == 0),
                                stop=(m == mg - 1 and h == 1),
                            )
                        t_idx += 2
                    scratch = rpool.tile([16, CW], _dt.float32)
                    nc.vector.tensor_tensor(
                        scratch[:], T[:], SJP[:, g * CW:(g + 1) * CW],
                        op=mybir.AluOpType.mult,
                    )
                    nc.vector.tensor_reduce(
                        ACC[:, g:g + 1], scratch[:],
                        axis=mybir.AxisListType.X, op=mybir.AluOpType.add,
                    )
            nc.sync.dma_start(acc_d[:], ACC[:])
    nc.compile()
    return nc


_NC = None


def _get_program():
    global _NC
    if _NC is None:
        _NC = build_program()
    return _NC


def kernel(images, segmentations, ROIs):
    nc = _get_program()
    in_maps = build_inputs(images, segmentations)
    res = run_bass_kernel_spmd(nc, in_maps, list(range(NCORES)))
    total = np.float64(0.0)
    for core in res.results:
        total += np.asarray(core["ACC"], np.float64).sum()
    return np.float32(-LOSS_WEIGHT * total / N_IMG)


# revision 7
# speedup vs baseline: 12.2466x; 12.2466x over previous
"""DenseCRF Gaussian-kernel loss on 8 TRN2 NeuronCores.

loss = -W/N * sum_n sum_ij exp(-0.5||f_i-f_j||^2) * (S^T S)_ij

Decomposition (per image, P=6400 pixels, f in R^5):
  exp arg  = f_i.f_j - 0.5|f_i|^2 - 0.5|f_j|^2   -> one 9-deep fp16 matmul
             (rows: 5 features + ones + hi/lo split of -0.5|f|^2), so the
             exp needs no per-row bias and tiles can share ACT ops freely.
  sum_ij W_ij G_ij = sum_k (S_k^T W S_k): per W tile, accumulate
             T += (w * S_rows)^T @ W on the PE, then one multiply+reduce
             against S_cols on the DVE per column group.
  Symmetry W_ij = W_ji halves the work: strictly-upper 128x512 tiles at
             weight 2 (folded into the S^T matmul weights), and the diagonal
             handled as 2x2-block squares at weight 1 plus 128x256
             upper-within-band tiles at weight 2.

Sharding: cores 2n, 2n+1 split image n's triangle (strict tiles by row
parity, diagonal bands by band parity); all cores run one identical program
over host-packed per-core operands.  Each core returns a [16, n_groups]
partial-sum block; the host reduces to the scalar.
"""

import os

# The Bass program executes through jax/PJRT on the axon-tunneled TRN2 cores;
# a JAX_PLATFORMS=cpu pin (common for running the jax reference) would silently
# reroute execution to a fake NRT.  Clear it before jax initializes.
if os.environ.get("JAX_PLATFORMS") == "cpu":
    del os.environ["JAX_PLATFORMS"]

import numpy as np
import ml_dtypes

import concourse.bacc as bacc
import concourse.bass as bass  # noqa: F401
import concourse.mybir as mybir
import concourse.tile as tile
from concourse.bass_utils import run_bass_kernel_spmd

N_IMG, K_CLS, H_IN, W_IN = 4, 16, 160, 160
HO = WO = 80
P = HO * WO               # 6400 pixels after 2x downscale
SIGMA_RGB = 15.0
SIGMA_XY = 50.0           # 100 * scale_factor 0.5
LOSS_WEIGHT = 2e-9
NCORES = 8

_dt = mybir.dt
_BF16 = ml_dtypes.bfloat16

# ---------------------------------------------------------------------------
# Schedule: identical program structure for every core.  Column data lives in
# packed per-core BJP/SJP arrays: 11 strict 512-col groups (global columns of
# C=1..11), then 6 per-core band slots of 512, then one 256 remainder slot.
#
# group kinds:
#   strict C (C=1..11): C megas, each 2 tiles [128,512] sharing one x[128,1024]
#   band b (b=0..5):    3 megas of 2 tiles [128,256] (x[128,512]):
#                       left square -> T[:,0:256], right square + upper-right
#                       -> T[:,256:512]
#   rem:                13 megas of 2 tiles [128,256], all into T[:,0:256]
# ---------------------------------------------------------------------------

STRICT_OFF = 0                       # BJP col offset of strict group C: (C-1)*512
BAND_OFF = 11 * 512                  # band slot b: BAND_OFF + b*512
REM_OFF = BAND_OFF + 6 * 512         # 256-wide remainder slot
PJ = REM_OFF + 256                   # 8960 packed columns


def make_schedule():
    """List of groups; each: dict(sj_off, sj_w, megas=[mega...]).
    mega: dict(bj_off, bj_w, t_off, start, stop) covering 2 tiles."""
    groups = []
    for C in range(1, 12):
        off = (C - 1) * 512
        megas = [
            dict(bj_off=off, bj_w=512, t_off=0,
                 start=(m == 0), stop=(m == C - 1))
            for m in range(C)
        ]
        groups.append(dict(sj_off=off, sj_w=512, megas=megas))
    for b in range(6):
        off = BAND_OFF + b * 512
        megas = [
            dict(bj_off=off, bj_w=256, t_off=0, start=True, stop=True),
            dict(bj_off=off + 256, bj_w=256, t_off=256, start=True, stop=False),
            dict(bj_off=off + 256, bj_w=256, t_off=256, start=False, stop=True),
        ]
        groups.append(dict(sj_off=off, sj_w=512, megas=megas))
    megas = [
        dict(bj_off=REM_OFF, bj_w=256, t_off=0,
             start=(m == 0), stop=(m == 12))
        for m in range(13)
    ]
    groups.append(dict(sj_off=REM_OFF, sj_w=256, megas=megas))
    return groups


SCHEDULE = make_schedule()
NGROUPS = len(SCHEDULE)                               # 18
NTILE = 2 * sum(len(g["megas"]) for g in SCHEDULE)    # 194


def core_tiles(half):
    """Tile contents for core half h, aligned with the schedule's flat tile
    stream: list of (row_block, weight) or None (dummy)."""
    tiles = []
    for C in range(1, 12):                       # strict groups: rows r<4C, parity h
        rows = [r for r in range(4 * C) if r % 2 == half]
        assert len(rows) == 2 * C
        tiles += [(r, 2.0) for r in rows]
    for b in range(6):                           # band C = 2b + half
        C = 2 * b + half
        r0 = 4 * C
        tiles += [(r0, 1.0), (r0 + 1, 1.0)]          # left square
        tiles += [(r0 + 2, 1.0), (r0 + 3, 1.0)]      # right square
        tiles += [(r0, 2.0), (r0 + 1, 2.0)]          # upper-right
    rows = [r for r in range(48) if r % 2 == half]   # remainder strict
    tiles += [(r, 2.0) for r in rows]
    tiles += [(48 + half, 1.0), None]                # remainder diag + pad
    assert len(tiles) == NTILE
    return tiles


def band_cols(half):
    """Global column ranges feeding the packed band + remainder slots."""
    rngs = [(512 * (2 * b + half), 512 * (2 * b + half) + 512) for b in range(6)]
    rngs.append((6144, 6400))
    return rngs


# ---------------------------------------------------------------------------
# Host-side feature/segmentation prep
# ---------------------------------------------------------------------------

def _pool2x2(x):
    # torch bilinear align_corners=False at exact 2x = 2x2 average, in the
    # reference's evaluation order.
    r = x[..., 0::2, :] * 0.5 + x[..., 1::2, :] * 0.5
    return r[..., 0::2] * 0.5 + r[..., 1::2] * 0.5


def _features(img):
    """img [3,160,160] f32 -> (AI [9,P] f16, BJ [9,P] f16).

    exp arg for pair (i,j) = sum_d AI[d,i]*BJ[d,j]:
      AI = [f(5), 1, 1, shi, slo];  BJ = [f(5), shi, slo, 1, 1]
    with shi+slo an fp16 hi/lo split of -0.5|f|^2 so the diagonal cancels to
    ~1e-4 even though a single fp16 value could only hold it to ~0.2.
    """
    sub = img[:, ::2, ::2]                                  # nearest resize
    rgb = sub.reshape(3, P) / SIGMA_RGB
    yy, xx = np.meshgrid(np.arange(HO, dtype=np.float32),
                         np.arange(WO, dtype=np.float32), indexing="ij")
    pos = np.stack([xx.ravel(), yy.ravel()]) / SIGMA_XY     # [2,P]
    f16 = np.concatenate([pos, rgb], 0).astype(np.float16)  # [5,P]
    fh = f16.astype(np.float64)
    s = -0.5 * (fh * fh).sum(0)                             # [P] exact
    shi = s.astype(np.float16)
    slo = (s - shi.astype(np.float64)).astype(np.float16)
    ones = np.ones(P, np.float16)
    AI = np.concatenate([f16, ones[None], ones[None], shi[None], slo[None]])
    BJ = np.concatenate([f16, shi[None], slo[None], ones[None], ones[None]])
    return AI, BJ


def _pack_core(AI, BJ, S, half):
    tiles = core_tiles(half)
    aip = np.zeros((9, NTILE * 128), np.float16)
    sitp = np.zeros((128, NTILE * 16), np.float32)
    for t, ent in enumerate(tiles):
        if ent is None:
            aip[:, t * 128:(t + 1) * 128] = AI[:, 0:128]   # safe exp args
            continue
        r, w = ent
        aip[:, t * 128:(t + 1) * 128] = AI[:, r * 128:(r + 1) * 128]
        sitp[:, t * 16:(t + 1) * 16] = w * S[:, r * 128:(r + 1) * 128].T
    bjp = np.zeros((9, PJ), np.float16)
    sjp = np.zeros((16, PJ), np.float32)
    bjp[:, 0:BAND_OFF] = BJ[:, 512:6144]        # strict groups C=1..11
    sjp[:, 0:BAND_OFF] = S[:, 512:6144]
    for slot, (a, b) in enumerate(band_cols(half)):
        off = BAND_OFF + slot * 512
        bjp[:, off:off + (b - a)] = BJ[:, a:b]
        sjp[:, off:off + (b - a)] = S[:, a:b]
    return {"AIP": aip, "BJP": bjp, "SITP": sitp.astype(_BF16), "SJP": sjp}


def build_inputs(images, segmentations):
    """FULL inputs -> per-core in_maps (cores 2n, 2n+1 share image n)."""
    in_maps = []
    for n in range(N_IMG):
        AI, BJ = _features(np.asarray(images[n], np.float32))
        S = _pool2x2(np.asarray(segmentations[n], np.float32)).reshape(K_CLS, P)
        for half in range(2):
            in_maps.append(_pack_core(AI, BJ, S, half))
    return in_maps


# ---------------------------------------------------------------------------
# Device program
# ---------------------------------------------------------------------------

def build_program(repeat=1):
    # repeat>1 re-runs the (idempotent) compute body back-to-back inside one
    # NEFF — used only by the benchmark to difference away dispatch overhead.
    nc = bacc.Bacc("TRN2", target_bir_lowering=False, debug=False)
    aip_d = nc.dram_tensor("AIP", (9, NTILE * 128), _dt.float16, kind="ExternalInput")
    bjp_d = nc.dram_tensor("BJP", (9, PJ), _dt.float16, kind="ExternalInput")
    sitp_d = nc.dram_tensor("SITP", (128, NTILE * 16), _dt.bfloat16, kind="ExternalInput")
    sjp_d = nc.dram_tensor("SJP", (16, PJ), _dt.float32, kind="ExternalInput")
    acc_d = nc.dram_tensor("ACC", (16, NGROUPS), _dt.float32, kind="ExternalOutput")

    with tile.TileContext(nc) as tc:
        with (
            tc.tile_pool(name="const", bufs=1) as cpool,
            tc.tile_pool(name="w", bufs=4) as wpool,
            tc.tile_pool(name="red", bufs=2) as rpool,
            tc.tile_pool(name="xps", bufs=3, space="PSUM") as xpool,
            tc.tile_pool(name="tps", bufs=2, space="PSUM") as tpool,
        ):
            AIP = cpool.tile([9, NTILE * 128], _dt.float16)
            BJP = cpool.tile([9, PJ], _dt.float16)
            SITP = cpool.tile([128, NTILE * 16], _dt.bfloat16)
            SJP = cpool.tile([16, PJ], _dt.float32)
            ACC = cpool.tile([16, NGROUPS], _dt.float32)
            nc.sync.dma_start(AIP[:], aip_d[:])
            nc.sync.dma_start(BJP[:], bjp_d[:])
            nc.sync.dma_start(SITP[:], sitp_d[:])
            nc.sync.dma_start(SJP[:], sjp_d[:])

            for _rep in range(repeat):
                t_idx = 0
                for gi, grp in enumerate(SCHEDULE):
                    T = tpool.tile([16, grp["sj_w"]], _dt.float32)
                    for mega in grp["megas"]:
                        bw = mega["bj_w"]
                        x = xpool.tile([128, 2 * bw], _dt.float32)
                        for h in range(2):
                            nc.tensor.matmul(
                                x[:, h * bw:(h + 1) * bw],
                                AIP[:, (t_idx + h) * 128:(t_idx + h + 1) * 128],
                                BJP[:, mega["bj_off"]:mega["bj_off"] + bw],
                                start=True, stop=True,
                            )
                        w = wpool.tile([128, 2 * bw], _dt.bfloat16)
                        nc.scalar.activation(w[:], x[:], mybir.ActivationFunctionType.Exp)
                        to = mega["t_off"]
                        for h in range(2):
                            nc.tensor.matmul(
                                T[:, to:to + bw],
                                SITP[:, (t_idx + h) * 16:(t_idx + h + 1) * 16],
                                w[:, h * bw:(h + 1) * bw],
                                start=(mega["start"] and h == 0),
                                stop=(mega["stop"] and h == 1),
                            )
                        t_idx += 2
                    scratch = rpool.tile([16, grp["sj_w"]], _dt.float32)
                    nc.vector.tensor_tensor(
                        scratch[:], T[:],
                        SJP[:, grp["sj_off"]:grp["sj_off"] + grp["sj_w"]],
                        op=mybir.AluOpType.mult,
                    )
                    nc.vector.tensor_reduce(
                        ACC[:, gi:gi + 1], scratch[:],
                        axis=mybir.AxisListType.X, op=mybir.AluOpType.add,
                    )
            nc.sync.dma_start(acc_d[:], ACC[:])
    nc.compile()
    return nc


_NC = None


def _get_program():
    global _NC
    if _NC is None:
        _NC = build_program()
    return _NC


def kernel(images, segmentations, ROIs):
    nc = _get_program()
    in_maps = build_inputs(images, segmentations)
    res = run_bass_kernel_spmd(nc, in_maps, list(range(NCORES)))
    total = np.float64(0.0)
    for core in res.results:
        total += np.asarray(core["ACC"], np.float64).sum()
    return np.float32(-LOSS_WEIGHT * total / N_IMG)


# revision 9
# speedup vs baseline: 22.2051x; 1.8132x over previous
"""DenseCRF Gaussian-kernel loss on 8 TRN2 NeuronCores.

loss = -W/N * sum_n sum_ij exp(-0.5||f_i-f_j||^2) * (S^T S)_ij

Decomposition (per image, P=6400 pixels, f in R^5):
  exp arg  = f_i.f_j - 0.5|f_i|^2 - 0.5|f_j|^2   -> one 9-deep fp16 matmul
             (rows: 5 features + ones + hi/lo split of -0.5|f|^2), so the
             exp needs no per-row bias and tiles can share ACT ops freely.
  sum_ij W_ij G_ij = sum_k (S_k^T W S_k): per W tile, accumulate
             T += (w * S_rows)^T @ W on the PE, then multiply+reduce against
             S_cols on the DVE per column group.
  Symmetry W_ij = W_ji halves the work: strictly-upper 128x512 tiles at
             weight 2 (folded into the S^T matmul weights), the diagonal as
             2x2-block squares at weight 1 plus 128x256 upper-within-band
             tiles at weight 2.
  The S^T@W matmuls have only 16 output rows, so they are packed 4 (or 2) at
  a time into disjoint 32-column strips of the PE array via
  tile_position=(0,32j) — concurrent strips make the AS pass ~4x cheaper.

Sharding: cores 2n, 2n+1 split image n's triangle (strict tiles by row
parity, diagonal bands by band parity); all cores run one identical program
over host-packed per-core operands.  Each core returns a [128, n_groups]
partial-sum block; the host reduces to the scalar.
"""

import os

# The Bass program executes through jax/PJRT on the axon-tunneled TRN2 cores;
# a JAX_PLATFORMS=cpu pin (common for running the jax reference) would silently
# reroute execution to a fake NRT.  Clear it before jax initializes.
if os.environ.get("JAX_PLATFORMS") == "cpu":
    del os.environ["JAX_PLATFORMS"]

import numpy as np
import ml_dtypes

import concourse.bacc as bacc
import concourse.bass as bass  # noqa: F401
import concourse.mybir as mybir
import concourse.tile as tile
from concourse.bass_utils import run_bass_kernel_spmd

N_IMG, K_CLS, H_IN, W_IN = 4, 16, 160, 160
HO = WO = 80
P = HO * WO               # 6400 pixels after 2x downscale
SIGMA_RGB = 15.0
SIGMA_XY = 50.0           # 100 * scale_factor 0.5
LOSS_WEIGHT = 2e-9
NCORES = 8

_dt = mybir.dt
_BF16 = ml_dtypes.bfloat16

# ---------------------------------------------------------------------------
# Schedule: identical program structure for every core.  Packed BJP/SJP
# column layout: 11 strict 512-col groups (global columns of C=1..11), then
# 6 per-core band slots of 512, then one 256 remainder slot.
# ---------------------------------------------------------------------------

STRICT_OFF = 0                       # strict group C -> offset (C-1)*512
BAND_OFF = 11 * 512                  # band slot b -> BAND_OFF + b*512
REM_OFF = BAND_OFF + 6 * 512         # 256-wide remainder slot
PJ = REM_OFF + 256                   # 8960 packed columns


def make_schedule():
    """Groups of: megas (2 tiles each: bj slice, per-tile strip + start/stop)
    and ttr ops (partition base/rows + SJ slice) consuming T afterwards."""
    groups = []
    for C in range(1, 12):           # strict: 2C tiles round-robin on 4 strips
        off = (C - 1) * 512
        nt = 2 * C
        megas = []
        for m in range(C):
            tiles = []
            for h in range(2):
                it = 2 * m + h
                tiles.append(dict(strip=it % 4, start=it < 4, stop=it + 4 >= nt))
            megas.append(dict(bj_off=off, bj_w=512, tiles=tiles))
        if nt >= 4:
            ttr = [dict(pbase=0, rows=128, sj_off=off, sj_w=512)]
        else:                         # C=1: strips 2,3 never written
            ttr = [dict(pbase=32 * j, rows=32, sj_off=off, sj_w=512)
                   for j in range(nt)]
        groups.append(dict(t_w=512, megas=megas, ttr=ttr))
    for b in range(6):               # bands: strip0 = left square (2 tiles),
        off = BAND_OFF + b * 512     # strip1 = right square + upper (4 tiles)
        megas = [
            dict(bj_off=off, bj_w=256,
                 tiles=[dict(strip=0, start=True, stop=False),
                        dict(strip=0, start=False, stop=True)]),
            dict(bj_off=off + 256, bj_w=256,
                 tiles=[dict(strip=1, start=True, stop=False),
                        dict(strip=1, start=False, stop=False)]),
            dict(bj_off=off + 256, bj_w=256,
                 tiles=[dict(strip=1, start=False, stop=False),
                        dict(strip=1, start=False, stop=True)]),
        ]
        ttr = [dict(pbase=0, rows=32, sj_off=off, sj_w=256),
               dict(pbase=32, rows=32, sj_off=off + 256, sj_w=256)]
        groups.append(dict(t_w=256, megas=megas, ttr=ttr))
    megas = []                        # remainder: 26 tiles on 4 strips
    for m in range(13):
        tiles = []
        for h in range(2):
            it = 2 * m + h
            tiles.append(dict(strip=it % 4, start=it < 4, stop=it + 4 >= 26))
        megas.append(dict(bj_off=REM_OFF, bj_w=256, tiles=tiles))
    groups.append(dict(t_w=256, megas=megas,
                       ttr=[dict(pbase=0, rows=128, sj_off=REM_OFF, sj_w=256)]))
    return groups


SCHEDULE = make_schedule()
NGROUPS = len(SCHEDULE)                               # 18
NTILE = 2 * sum(len(g["megas"]) for g in SCHEDULE)    # 194


def core_tiles(half):
    """Tile contents for core half h, aligned with the schedule's flat tile
    stream: list of (row_block, weight) or None (dummy)."""
    tiles = []
    for C in range(1, 12):                       # strict groups: rows r<4C, parity h
        rows = [r for r in range(4 * C) if r % 2 == half]
        assert len(rows) == 2 * C
        tiles += [(r, 2.0) for r in rows]
    for b in range(6):                           # band C = 2b + half
        C = 2 * b + half
        r0 = 4 * C
        tiles += [(r0, 1.0), (r0 + 1, 1.0)]          # left square
        tiles += [(r0 + 2, 1.0), (r0 + 3, 1.0)]      # right square
        tiles += [(r0, 2.0), (r0 + 1, 2.0)]          # upper-right
    rows = [r for r in range(48) if r % 2 == half]   # remainder strict
    tiles += [(r, 2.0) for r in rows]
    tiles += [(48 + half, 1.0), None]                # remainder diag + pad
    assert len(tiles) == NTILE
    return tiles


def band_cols(half):
    """Global column ranges feeding the packed band + remainder slots."""
    rngs = [(512 * (2 * b + half), 512 * (2 * b + half) + 512) for b in range(6)]
    rngs.append((6144, 6400))
    return rngs


# ---------------------------------------------------------------------------
# Host-side feature/segmentation prep
# ---------------------------------------------------------------------------

def _pool2x2(x):
    # torch bilinear align_corners=False at exact 2x = 2x2 average, in the
    # reference's evaluation order.
    r = x[..., 0::2, :] * 0.5 + x[..., 1::2, :] * 0.5
    return r[..., 0::2] * 0.5 + r[..., 1::2] * 0.5


def _features(img):
    """img [3,160,160] f32 -> (AI [9,P] f16, BJ [9,P] f16).

    exp arg for pair (i,j) = sum_d AI[d,i]*BJ[d,j]:
      AI = [f(5), 1, 1, shi, slo];  BJ = [f(5), shi, slo, 1, 1]
    with shi+slo an fp16 hi/lo split of -0.5|f|^2 so the diagonal cancels to
    ~1e-4 even though a single fp16 value could only hold it to ~0.2.
    """
    sub = img[:, ::2, ::2]                                  # nearest resize
    rgb = sub.reshape(3, P) / SIGMA_RGB
    yy, xx = np.meshgrid(np.arange(HO, dtype=np.float32),
                         np.arange(WO, dtype=np.float32), indexing="ij")
    pos = np.stack([xx.ravel(), yy.ravel()]) / SIGMA_XY     # [2,P]
    f16 = np.concatenate([pos, rgb], 0).astype(np.float16)  # [5,P]
    fh = f16.astype(np.float64)
    s = -0.5 * (fh * fh).sum(0)                             # [P] exact
    shi = s.astype(np.float16)
    slo = (s - shi.astype(np.float64)).astype(np.float16)
    ones = np.ones(P, np.float16)
    AI = np.concatenate([f16, ones[None], ones[None], shi[None], slo[None]])
    BJ = np.concatenate([f16, shi[None], slo[None], ones[None], ones[None]])
    return AI, BJ


def _pack_core(AI, BJ, S, half):
    tiles = core_tiles(half)
    aip = np.zeros((9, NTILE * 128), np.float16)
    sitp = np.zeros((128, NTILE * 32), np.float32)   # 32-wide slots, top half 0
    for t, ent in enumerate(tiles):
        if ent is None:
            aip[:, t * 128:(t + 1) * 128] = AI[:, 0:128]   # safe exp args
            continue
        r, w = ent
        aip[:, t * 128:(t + 1) * 128] = AI[:, r * 128:(r + 1) * 128]
        sitp[:, t * 32:t * 32 + 16] = w * S[:, r * 128:(r + 1) * 128].T
    bjp = np.zeros((9, PJ), np.float16)
    sjp = np.zeros((128, PJ), np.float32)            # S replicated at 0/32/64/96
    bjp[:, 0:BAND_OFF] = BJ[:, 512:6144]             # strict groups C=1..11
    for j in range(4):
        sjp[32 * j:32 * j + 16, 0:BAND_OFF] = S[:, 512:6144]
    for slot, (a, b) in enumerate(band_cols(half)):
        off = BAND_OFF + slot * 512
        bjp[:, off:off + (b - a)] = BJ[:, a:b]
        for j in range(4):
            sjp[32 * j:32 * j + 16, off:off + (b - a)] = S[:, a:b]
    return {"AIP": aip, "BJP": bjp, "SITP": sitp.astype(_BF16), "SJP": sjp}


def build_inputs(images, segmentations):
    """FULL inputs -> per-core in_maps (cores 2n, 2n+1 share image n)."""
    in_maps = []
    for n in range(N_IMG):
        AI, BJ = _features(np.asarray(images[n], np.float32))
        S = _pool2x2(np.asarray(segmentations[n], np.float32)).reshape(K_CLS, P)
        for half in range(2):
            in_maps.append(_pack_core(AI, BJ, S, half))
    return in_maps


# ---------------------------------------------------------------------------
# Device program
# ---------------------------------------------------------------------------

def build_program(repeat=1):
    # repeat>1 re-runs the (idempotent) compute body back-to-back inside one
    # NEFF — used only by the benchmark to difference away dispatch overhead.
    nc = bacc.Bacc("TRN2", target_bir_lowering=False, debug=False)
    aip_d = nc.dram_tensor("AIP", (9, NTILE * 128), _dt.float16, kind="ExternalInput")
    bjp_d = nc.dram_tensor("BJP", (9, PJ), _dt.float16, kind="ExternalInput")
    sitp_d = nc.dram_tensor("SITP", (128, NTILE * 32), _dt.bfloat16, kind="ExternalInput")
    sjp_d = nc.dram_tensor("SJP", (128, PJ), _dt.float32, kind="ExternalInput")
    acc_d = nc.dram_tensor("ACC", (128, NGROUPS), _dt.float32, kind="ExternalOutput")

    with tile.TileContext(nc) as tc:
        with (
            tc.tile_pool(name="const", bufs=1) as cpool,
            tc.tile_pool(name="w", bufs=4) as wpool,
            tc.tile_pool(name="red", bufs=2) as rpool,
            tc.tile_pool(name="xps", bufs=3, space="PSUM") as xpool,
            tc.tile_pool(name="tps", bufs=2, space="PSUM") as tpool,
        ):
            AIP = cpool.tile([9, NTILE * 128], _dt.float16)
            BJP = cpool.tile([9, PJ], _dt.float16)
            SITP = cpool.tile([128, NTILE * 32], _dt.bfloat16)
            SJP = cpool.tile([128, PJ], _dt.float32)
            ACC = cpool.tile([128, NGROUPS], _dt.float32)
            nc.sync.dma_start(AIP[:], aip_d[:])
            nc.sync.dma_start(BJP[:], bjp_d[:])
            nc.sync.dma_start(SITP[:], sitp_d[:])
            nc.sync.dma_start(SJP[:], sjp_d[:])
            nc.gpsimd.memset(ACC[:], 0.0)

            for _rep in range(repeat):
                t_idx = 0
                for gi, grp in enumerate(SCHEDULE):
                    tw = grp["t_w"]
                    T = tpool.tile([128, tw], _dt.float32)
                    for mega in grp["megas"]:
                        bw = mega["bj_w"]
                        x = xpool.tile([128, 2 * bw], _dt.float32)
                        for h in range(2):
                            nc.tensor.matmul(
                                x[:, h * bw:(h + 1) * bw],
                                AIP[:, (t_idx + h) * 128:(t_idx + h + 1) * 128],
                                BJP[:, mega["bj_off"]:mega["bj_off"] + bw],
                                start=True, stop=True,
                            )
                        w = wpool.tile([128, 2 * bw], _dt.bfloat16)
                        nc.scalar.activation(w[:], x[:], mybir.ActivationFunctionType.Exp)
                        for h in range(2):
                            td = mega["tiles"][h]
                            sp = td["strip"]
                            nc.tensor.matmul(
                                T[32 * sp:32 * sp + 32, :bw],
                                SITP[:, (t_idx + h) * 32:(t_idx + h + 1) * 32],
                                w[:, h * bw:(h + 1) * bw],
                                start=td["start"], stop=td["stop"],
                                tile_position=(0, 32 * sp),
                                # strip chains share a bank on disjoint
                                # partitions; the sim's zero-region conflict
                                # check doesn't model the partition split
                                skip_group_check=True,
                            )
                        t_idx += 2
                    scratch = rpool.tile([128, tw], _dt.float32)
                    for op in grp["ttr"]:
                        pb, rows = op["pbase"], op["rows"]
                        nc.vector.tensor_tensor(
                            scratch[pb:pb + rows, :],
                            T[pb:pb + rows, :],
                            SJP[pb:pb + rows, op["sj_off"]:op["sj_off"] + op["sj_w"]],
                            op=mybir.AluOpType.mult,
                        )
                        nc.vector.tensor_reduce(
                            ACC[pb:pb + rows, gi:gi + 1],
                            scratch[pb:pb + rows, :],
                            axis=mybir.AxisListType.X, op=mybir.AluOpType.add,
                        )
            nc.sync.dma_start(acc_d[:], ACC[:])
    nc.compile()
    return nc


_NC = None


def _get_program():
    global _NC
    if _NC is None:
        _NC = build_program()
    return _NC


def kernel(images, segmentations, ROIs):
    nc = _get_program()
    in_maps = build_inputs(images, segmentations)
    res = run_bass_kernel_spmd(nc, in_maps, list(range(NCORES)))
    total = np.float64(0.0)
    for core in res.results:
        total += np.asarray(core["ACC"], np.float64).sum()
    return np.float32(-LOSS_WEIGHT * total / N_IMG)
